# revision 1
# baseline (speedup 1.0000x reference)
"""Trainium2 Bass kernel for a pre-LN transformer block (dense_transformer).

Reference computation (fp32, per batch element):
    x = x + Attn(LN1(x));  x = x + MLP(LN2(x))
with 16-head causal ALiBi attention (S=2048, D=2048) and a 4*D GELU MLP.

Distribution: 4 batches x 2-way head/tensor parallel = 8 cores.
Core c handles batch c//2 with pair-rank r=c%2:
  - attention: 8 local heads (r*8..r*8+7), all 2048 query positions. Scores
    are computed transposed [j(key) x i(query)] so the ALiBi +s_h*j term rides
    the ACT-exp per-partition bias and the -s_h*i term is a K=3 seed matmul
    (bf16 hi/lo/lolo rows reconstruct the fp32 column bias in PSUM). The
    combined exp argument qk/sqrt(hd) + s_h*(j-i) is <= O(1), so no running
    max is needed; per-query softmax scale factors cancel in the normalize.
  - softmax denominators via ones-lhsT matmuls accumulated alongside AV;
    normalization is fused into the AV PSUM->SBUF copy using a K=1 broadcast
    matmul of the reciprocal row.
  - a pair AllToAll swaps attention halves so each core owns 1024 tokens for
    the output projection, residual, LN2 and MLP (full 4*D hidden).
All per-core variation (weight slices, ALiBi slopes, token offsets) is input
DATA; the instruction stream is identical on all 8 cores (SPMD).

STATUS: algorithm validated end-to-end on an 8-core CoreSim run (S=512
scaled config, nonzero biases/LN params, pair AllGather included): max rel
err 8.7e-4 vs a float64 numpy reference (pre-legalization build). The
walrus in this container encodes at most ONE sync wait per instruction, so
_legalize_waits() splits every multi-wait instruction into wait-only
EventSemaphore instructions inserted immediately before it on the same
engine stream — order-preserving, so semantics are unchanged. CoreSim
cannot execute wait-only EventSemaphore instructions (its add_fake_sem_
updates assert), so post-legalization validation is walrus/HW-only:
verify with check_waits.py, then test.py.
"""

import os
import sys

for _p in ("/opt/trn_rl_repo", "/opt/trn_rl_repo/concourse"):
    if os.path.isdir(_p) and _p not in sys.path:
        sys.path.append(_p)

import numpy as np
import ml_dtypes

import concourse.bass as bass
import concourse.mybir as mybir
import concourse.tile as tile
from contextlib import ExitStack

BF16 = mybir.dt.bfloat16
F32 = mybir.dt.float32
AF = mybir.ActivationFunctionType
ALU = mybir.AluOpType

REAL_CFG = dict(S=2048, D=2048, F=8192, H=16, GELU="Gelu")
LN_EPS = 1e-5
NEG = -1.0e6  # causal mask additive value (pre-exp)


def _cfg_derived(cfg):
    S, D, F, H = cfg["S"], cfg["D"], cfg["F"], cfg["H"]
    d = dict(cfg)
    d["HL"] = H // 2              # local heads per core
    d["HLW"] = d["HL"] * 128      # local head width (vd)
    d["DT"] = D // 128
    d["ST"] = S // 128
    d["QW"] = 512                 # q-chunk width (asserted below)
    d["CQ"] = S // 512
    d["OWN"] = S // 2
    d["OTT"] = d["OWN"] // 128
    d["FT"] = F // 128
    d["VDT"] = H
    assert S % 512 == 0 and D % 512 == 0 and F % 512 == 0
    return d


# ------------------------------------------------------------ host prep ---


def _bf(x):
    return np.asarray(x, np.float32).astype(ml_dtypes.bfloat16)


def _split3(v):
    """Split fp32 array (last axis vectors) into 3 bf16 rows summing to it."""
    v = np.asarray(v, np.float32)
    r0 = v.astype(ml_dtypes.bfloat16)
    rem = v - r0.astype(np.float32)
    r1 = rem.astype(ml_dtypes.bfloat16)
    r2 = (rem - r1.astype(np.float32)).astype(ml_dtypes.bfloat16)
    return np.stack([r0, r1, r2])


def make_core_inputs(cfg, inputs, core):
    c = _cfg_derived(cfg)
    S, D, F, H, HL, ST = c["S"], c["D"], c["F"], c["H"], c["HL"], c["ST"]
    HLW, OWN, FT = c["HLW"], c["OWN"], c["FT"]
    b, r = core // 2, core % 2
    hd = 128
    f32 = np.float32

    x = np.asarray(inputs["x"][b], f32)
    g1 = np.asarray(inputs["ln1_w"], f32)
    c1 = np.asarray(inputs["ln1_b"], f32)
    g2 = np.asarray(inputs["ln2_w"], f32)
    c2 = np.asarray(inputs["ln2_b"], f32)
    Wqkv = np.asarray(inputs["Wqkv"], f32)
    bqkv = np.asarray(inputs["bqkv"], f32)
    Wo = np.asarray(inputs["Wo"], f32)
    bo = np.asarray(inputs["bo"], f32)
    W1 = np.asarray(inputs["W1"], f32)
    b1 = np.asarray(inputs["b1"], f32)
    W2 = np.asarray(inputs["W2"], f32)
    b2 = np.asarray(inputs["b2"], f32)
    slopes = np.asarray(inputs["slopes"], f32)

    Wq, Wk, Wv = Wqkv[:D], Wqkv[D:2 * D], Wqkv[2 * D:]
    bq, bk, bv = bqkv[:D], bqkv[D:2 * D], bqkv[2 * D:]

    lo, hi = r * HLW, (r + 1) * HLW
    sc = 1.0 / np.sqrt(hd)

    Wq_l = Wq[lo:hi] * g1[None, :]
    Wk_l = Wk[lo:hi] * g1[None, :]
    Wv_l = Wv[lo:hi] * g1[None, :]
    qb = (Wq[lo:hi] @ c1 + bq[lo:hi]) * sc
    kb = Wk[lo:hi] @ c1 + bk[lo:hi]
    cbo = Wo @ (Wv @ c1 + bv) + bo          # v-bias + bo folded constant [D]

    W1p = W1 * g2[None, :]
    b1p = W1 @ c2 + b1

    heads = np.arange(r * HL, r * HL + HL)
    sl = slopes[heads]
    jpos = np.arange(S, dtype=f32)

    expbias = np.zeros((128, HL * ST), f32)
    for h in range(HL):
        for jt in range(ST):
            expbias[:, h * ST + jt] = sl[h] * (jt * 128 + jpos[:128])

    seed = np.zeros((3, HL * S), f32)
    for h in range(HL):
        seed[:, h * S:(h + 1) * S] = _split3(-sl[h] * jpos)

    masks = np.zeros((128, 4 * 512), f32)
    jj = np.arange(128)[:, None]
    ii = np.arange(512)[None, :]
    for m in range(4):
        masks[:, m * 512:(m + 1) * 512] = np.where(m * 128 + jj <= ii, 0.0, NEG)

    return {
        "x_full": np.ascontiguousarray(x),
        "x_own": np.ascontiguousarray(x[r * OWN:(r + 1) * OWN]),
        "wq_t": np.ascontiguousarray(_bf(Wq_l.T * sc)),
        "wk_t": np.ascontiguousarray(_bf(Wk_l.T)),
        "wv_t": np.ascontiguousarray(_bf(Wv_l.T)),
        "wo_t": np.ascontiguousarray(_bf(Wo.T)),
        "w1_t": np.ascontiguousarray(_bf(W1p.T)),
        "w2_t": np.ascontiguousarray(_bf(W2.T)),
        "qb": np.ascontiguousarray(qb.reshape(HL, 128).T),
        "kb": np.ascontiguousarray(kb.reshape(HL, 128).T),
        "b1c": np.ascontiguousarray(b1p.reshape(FT, 128).T),
        "obias": np.ascontiguousarray(_split3(cbo)[:2]),
        "b2row": np.ascontiguousarray(_split3(b2)[:2]),
        "expbias": expbias,
        "seed": np.ascontiguousarray(seed.astype(ml_dtypes.bfloat16)),
        "masks": np.ascontiguousarray(masks.astype(ml_dtypes.bfloat16)),
        "ident": np.eye(128, dtype=f32).astype(ml_dtypes.bfloat16),
        "sel": np.ascontiguousarray(
            np.repeat((np.arange(2) == r).astype(f32)[None, :], 128, axis=0)),
    }


# ------------------------------------------------------------ the kernel ---


def build_kernel(cfg):
    c = _cfg_derived(cfg)
    S, D, F = c["S"], c["D"], c["F"]
    HL, HLW, DT, ST = c["HL"], c["HLW"], c["DT"], c["ST"]
    CQ, QW, OWN, OTT, FT, VDT = (c["CQ"], c["QW"], c["OWN"], c["OTT"],
                                 c["FT"], c["VDT"])
    GELU = getattr(AF, cfg["GELU"])

    # Single SWDGE sem lane: every DMA rides one FIFO queue (qPoolDynamic)
    # anyway, but Tile's default 8-lane round-robin sem assignment makes
    # slot-reuse DMAs wait on several DMASW sems at once, and the DMA ISA
    # encodes at most 2 waits (walrus "Too many sync wait commands").
    import concourse.tile_sem_assignment as tsa
    tsa.NUM_SWDGE_GLOBAL_SEMS = 1

    nc = bass.Bass()

    def param(name, shape, dt):
        return nc.declare_dram_parameter(name, shape, dt, isOutput=False)

    x_full_d = param("x_full", [S, D], F32)
    x_own_d = param("x_own", [OWN, D], F32)
    wq_d = param("wq_t", [D, HLW], BF16)
    wk_d = param("wk_t", [D, HLW], BF16)
    wv_d = param("wv_t", [D, HLW], BF16)
    wo_d = param("wo_t", [D, D], BF16)
    w1_d = param("w1_t", [D, F], BF16)
    w2_d = param("w2_t", [F, D], BF16)
    qb_d = param("qb", [128, HL], F32)
    kb_d = param("kb", [128, HL], F32)
    b1c_d = param("b1c", [128, FT], F32)
    obias_d = param("obias", [2, D], BF16)
    b2row_d = param("b2row", [2, D], BF16)
    expb_d = param("expbias", [128, HL * ST], F32)
    seed_d = param("seed", [3, HL * S], BF16)
    masks_d = param("masks", [128, 4 * 512], BF16)
    ident_d = param("ident", [128, 128], BF16)
    sel_d = param("sel", [128, 2], F32)
    out_d = nc.declare_dram_parameter("out", [OWN, D], F32, isOutput=True)

    groups = [[0, 1], [2, 3], [4, 5], [6, 7]]


    with tile.TileContext(nc) as tc, ExitStack() as top:
        def dma(out_ap, in_ap):
            nc.gpsimd.dma_start(out_ap, in_ap)

        def dma_blk(sbuf_tile, dram_ap):
            """DMA DRAM [T*128, N] into SBUF [128, T*N] (block t at t*N)."""
            rows = dram_ap.shape[0]
            t = rows // 128
            dma(sbuf_tile[:].rearrange("p (t f) -> p t f", t=t),
                dram_ap.rearrange("(t p) f -> p t f", p=128))

        dram = top.enter_context(tc.tile_pool(name="dram", bufs=1,
                                              space="DRAM"))
        exch_in = dram.tile([2, HLW, OWN], BF16)
        ago = dram.tile([2, 2, HLW, OWN], BF16)
        x2buf = dram.tile([OWN, D], F32)
        gbuf = dram.tile([F, OWN], BF16)
        pool_a = top.enter_context(tc.tile_pool(name="slotA", bufs=1))
        agp = top.enter_context(tc.tile_pool(name="agp", bufs=3))
        es_b, es_c, es_d = ExitStack(), ExitStack(), ExitStack()
        const = top.enter_context(tc.tile_pool(name="const", bufs=1))
        ident = const.tile([128, 128], BF16)
        masks = const.tile([128, 4 * 512], BF16)
        expb = const.tile([128, HL * ST], F32)
        qb = const.tile([128, HL], F32)
        kb = const.tile([128, HL], F32)
        b1c = const.tile([128, FT], F32)
        ones3 = const.tile([3, 128], BF16)
        ones2 = const.tile([2, 128], BF16)
        onesc = const.tile([128, 1], BF16)
        onesr = const.tile([1, 128], BF16)
        epsc = const.tile([128, 1], F32)
        sel = const.tile([128, 2], F32)

        dma(ident[:], ident_d[:])
        dma(masks[:], masks_d[:])
        dma(expb[:], expb_d[:])
        dma(qb[:], qb_d[:])
        dma(kb[:], kb_d[:])
        dma(b1c[:], b1c_d[:])
        nc.vector.memset(ones3[:], 1.0)
        nc.vector.memset(ones2[:], 1.0)
        nc.vector.memset(onesc[:], 1.0)
        nc.vector.memset(onesr[:], 1.0)
        nc.vector.memset(epsc[:], LN_EPS)
        dma(sel[:], sel_d[:])

        # ---- LayerNorm helper (normalized output only; w/b pre-folded) ---
        LNG = D // 512 if D >= 512 else 1

        def layernorm_tile(stat, scratch, xt, out_bf):
            st6 = stat.tile([128, 6 * LNG], F32)
            for g in range(LNG):
                nc.vector.bn_stats(st6[:, 6 * g:6 * (g + 1)],
                                   xt[:, 512 * g:512 * (g + 1)])
            ag = stat.tile([128, 2], F32)
            nc.vector.bn_aggr(ag[:], st6[:])
            sd = stat.tile([128, 1], F32)
            nc.scalar.activation(sd[:], ag[:, 1:2], AF.Sqrt,
                                 bias=epsc[:], scale=1.0)
            r = stat.tile([128, 1], F32)
            nc.vector.reciprocal(r[:], sd[:])
            nc.vector.tensor_scalar(
                out_bf[:], xt[:], scalar1=ag[:, 0:1], scalar2=r[:],
                op0=ALU.subtract, op1=ALU.mult)

        # ---- phase A+B: LN1 + transpose into h_fm ------------------------
        h_fm = pool_a.tile([128, DT * S], BF16, tag="a")
        with ExitStack() as ph:
            xpool = ph.enter_context(tc.tile_pool(name="ln_x", bufs=2))
            stat = ph.enter_context(tc.tile_pool(name="ln_stat", bufs=8))
            scratch = ph.enter_context(tc.tile_pool(name="ln_sq", bufs=2))
            hbf = ph.enter_context(tc.tile_pool(name="ln_h", bufs=2))
            tps = ph.enter_context(
                tc.tile_pool(name="tps", bufs=6, space="PSUM"))
            for tt in range(ST):
                xt = xpool.tile([128, D], F32)
                dma(xt[:], x_full_d[tt * 128:(tt + 1) * 128, :])
                ht = hbf.tile([128, D], BF16)
                layernorm_tile(stat, scratch, xt, ht)
                for dt in range(DT):
                    tp = tps.tile([128, 128], BF16)
                    nc.tensor.transpose(
                        tp[:], ht[:, dt * 128:(dt + 1) * 128], ident[:])
                    nc.vector.tensor_copy(
                        h_fm[:, dt * S + tt * 128: dt * S + (tt + 1) * 128],
                        tp[:])

        # ---- phase C1: K and V projections -------------------------------
        pool_b = es_b.enter_context(tc.tile_pool(name="slotB", bufs=1))
        pool_c = es_c.enter_context(tc.tile_pool(name="slotC", bufs=1))
        pool_d = es_d.enter_context(tc.tile_pool(name="slotD", bufs=1))
        k_sb = pool_b.tile([128, HL * S], BF16, tag="b")
        v_sb = pool_c.tile([128, ST * HLW], BF16, tag="c")
        with ExitStack() as ph:
            mps = ph.enter_context(
                tc.tile_pool(name="c1ps", bufs=2, space="PSUM"))
            wk_sb = pool_d.tile([128, DT * HLW], BF16, tag="d")
            dma_blk(wk_sb, wk_d.ap())
            for h in range(HL):
                for ch in range(CQ):
                    ps = mps.tile([128, QW], F32)
                    for dt in range(DT):
                        nc.tensor.matmul(
                            ps[:],
                            wk_sb[:, dt * HLW + h * 128:
                                  dt * HLW + (h + 1) * 128],
                            h_fm[:, dt * S + ch * QW: dt * S + (ch + 1) * QW],
                            start=(dt == 0), stop=(dt == DT - 1))
                    nc.vector.tensor_scalar_add(
                        k_sb[:, h * S + ch * QW: h * S + (ch + 1) * QW],
                        ps[:], kb[:, h:h + 1])
            wv_sb = pool_d.tile([128, DT * HLW], BF16, tag="d")
            dma_blk(wv_sb, wv_d.ap())
            VCW = min(512, HLW)
            for jt in range(ST):
                for vc in range(HLW // VCW):
                    ps = mps.tile([128, VCW], F32)
                    for dt in range(DT):
                        nc.tensor.matmul(
                            ps[:],
                            h_fm[:, dt * S + jt * 128: dt * S + (jt + 1) * 128],
                            wv_sb[:, dt * HLW + vc * VCW:
                                  dt * HLW + (vc + 1) * VCW],
                            start=(dt == 0), stop=(dt == DT - 1))
                    nc.vector.tensor_copy(
                        v_sb[:, jt * HLW + vc * VCW:
                             jt * HLW + (vc + 1) * VCW],
                        ps[:])

        # ---- phase C2: attention -----------------------------------------
        wq_sb = pool_d.tile([128, DT * HLW], BF16, tag="d")
        dma_blk(wq_sb, wq_d.ap())
        with ExitStack() as ph:
            qc_p = ph.enter_context(tc.tile_pool(name="qc", bufs=2))
            seed_p = ph.enter_context(tc.tile_pool(name="seedp", bufs=2))
            att_p = ph.enter_context(tc.tile_pool(name="att", bufs=3))
            bcn_p = ph.enter_context(tc.tile_pool(name="bcn", bufs=2))
            den_p = ph.enter_context(tc.tile_pool(name="den", bufs=2))
            oat_p = ph.enter_context(tc.tile_pool(name="oat", bufs=2))
            ps_q = ph.enter_context(
                tc.tile_pool(name="psq", bufs=1, space="PSUM"))
            ps_s = ph.enter_context(
                tc.tile_pool(name="pss", bufs=2, space="PSUM"))
            ps_a = ph.enter_context(
                tc.tile_pool(name="psa", bufs=2, space="PSUM"))
            ps_d = ph.enter_context(
                tc.tile_pool(name="psd", bufs=2, space="PSUM"))
            ps_b = ph.enter_context(
                tc.tile_pool(name="psb", bufs=1, space="PSUM"))
            for ct in range(CQ):
                njt = min(ST, (ct + 1) * (QW // 128))
                for h in range(HL):
                    pq = ps_q.tile([128, QW], F32)
                    for dt in range(DT):
                        nc.tensor.matmul(
                            pq[:],
                            wq_sb[:, dt * HLW + h * 128:
                                  dt * HLW + (h + 1) * 128],
                            h_fm[:, dt * S + ct * QW: dt * S + (ct + 1) * QW],
                            start=(dt == 0), stop=(dt == DT - 1))
                    qc = qc_p.tile([128, QW], BF16)
                    nc.vector.tensor_scalar_add(qc[:], pq[:], qb[:, h:h + 1])
                    seedt = seed_p.tile([3, QW], BF16)
                    dma(
                        seedt[:],
                        seed_d[:, h * S + ct * QW: h * S + (ct + 1) * QW])

                    pav = ps_a.tile([128, QW], F32)
                    pden = ps_d.tile([1, QW], F32)
                    for jt in range(njt):
                        pss = ps_s.tile([128, QW], F32)
                        nc.tensor.matmul(
                            pss[:], ones3[:], seedt[:],
                            start=True, stop=False)
                        nc.tensor.matmul(
                            pss[:],
                            k_sb[:, h * S + jt * 128: h * S + (jt + 1) * 128],
                            qc[:], start=False, stop=True)
                        m = jt - ct * (QW // 128)
                        if 0 <= m < 4:
                            nc.vector.tensor_add(
                                pss[:], pss[:],
                                masks[:, m * 512: m * 512 + QW])
                        at = att_p.tile([128, QW], BF16)
                        nc.scalar.activation(
                            at[:], pss[:], AF.Exp,
                            bias=expb[:, h * ST + jt: h * ST + jt + 1],
                            scale=1.0)
                        nc.tensor.matmul(
                            pav[:],
                            v_sb[:, jt * HLW + h * 128:
                                 jt * HLW + (h + 1) * 128],
                            at[:], start=(jt == 0), stop=(jt == njt - 1))
                        nc.tensor.matmul(
                            pden[:], onesc[:], at[:],
                            start=(jt == 0), stop=(jt == njt - 1))
                    dsb = den_p.tile([1, QW], F32)
                    nc.vector.tensor_copy(dsb[:], pden[:])
                    rec = den_p.tile([1, QW], F32)
                    nc.vector.reciprocal(rec[:], dsb[:])
                    recb = den_p.tile([1, QW], BF16)
                    nc.vector.tensor_copy(recb[:], rec[:])
                    pbc = ps_b.tile([128, QW], F32)
                    nc.tensor.matmul(pbc[:], onesr[:], recb[:],
                                     start=True, stop=True)
                    bcn = bcn_p.tile([128, QW], F32)
                    nc.vector.tensor_copy(bcn[:], pbc[:])
                    oat = oat_p.tile([128, QW], BF16)
                    nc.vector.scalar_tensor_tensor(
                        oat[:], pav[:], 1.0, bcn[:],
                        op0=ALU.mult, op1=ALU.mult)
                    for half in range(2):
                        a = max(ct * QW, half * OWN)
                        bnd = min((ct + 1) * QW, (half + 1) * OWN)
                        if a < bnd:
                            dma(
                                exch_in[half, h * 128:(h + 1) * 128,
                                        a - half * OWN: bnd - half * OWN],
                                oat[:, a - ct * QW: bnd - ct * QW])

        es_d.close()

        # ---- phase D: pair exchange --------------------------------------
        nc.gpsimd.collective_compute(
            "AllGather", ALU.bypass, replica_groups=groups,
            ins=[exch_in.opt()], outs=[ago.opt()])

        # ---- phase E: out-proj + residual + LN2 + transpose --------------
        h2_fm = pool_b.tile([128, DT * OWN], BF16, tag="b")
        with ExitStack() as ph:
            ob_p = ph.enter_context(tc.tile_pool(name="ob", bufs=1))
            xo_p = ph.enter_context(tc.tile_pool(name="xo", bufs=3))
            x2_p = ph.enter_context(tc.tile_pool(name="x2", bufs=2))
            h2_p = ph.enter_context(tc.tile_pool(name="h2", bufs=2))
            stat = ph.enter_context(tc.tile_pool(name="e_stat", bufs=8))
            scratch = ph.enter_context(tc.tile_pool(name="e_sq", bufs=2))
            ps_o = ph.enter_context(
                tc.tile_pool(name="pso", bufs=2, space="PSUM"))
            tps = ph.enter_context(
                tc.tile_pool(name="etps", bufs=6, space="PSUM"))

            attg = pool_c.tile([128, VDT * OWN], BF16, tag="c")
            for s in range(2):
                for h in range(HL):
                    g0 = agp.tile([128, OWN], BF16)
                    dma(g0[:], ago[s, 0, h * 128:(h + 1) * 128, :])
                    g1 = agp.tile([128, OWN], BF16)
                    dma(g1[:], ago[s, 1, h * 128:(h + 1) * 128, :])
                    t0 = agp.tile([128, OWN], BF16)
                    nc.vector.tensor_scalar_mul(t0[:], g0[:], sel[:, 0:1])
                    nc.vector.scalar_tensor_tensor(
                        attg[:, (s * HL + h) * OWN:(s * HL + h + 1) * OWN],
                        g1[:], sel[:, 1:2], t0[:],
                        op0=ALU.mult, op1=ALU.add)
            wo_sb = pool_a.tile([128, VDT * D], BF16, tag="a")
            dma_blk(wo_sb, wo_d.ap())
            obias = ob_p.tile([2, D], BF16)
            dma(obias[:], obias_d[:])
            for it in range(OTT):
                x2 = x2_p.tile([128, D], F32)
                for dc in range(D // 512):
                    po = ps_o.tile([128, 512], F32)
                    nc.tensor.matmul(
                        po[:], ones2[:], obias[:, dc * 512:(dc + 1) * 512],
                        start=True, stop=False)
                    for v in range(VDT):
                        nc.tensor.matmul(
                            po[:],
                            attg[:, v * OWN + it * 128:
                                 v * OWN + (it + 1) * 128],
                            wo_sb[:, v * D + dc * 512: v * D + (dc + 1) * 512],
                            start=False, stop=(v == VDT - 1))
                    xo = xo_p.tile([128, 512], F32)
                    dma(
                        xo[:],
                        x_own_d[it * 128:(it + 1) * 128,
                                dc * 512:(dc + 1) * 512])
                    nc.vector.tensor_add(
                        x2[:, dc * 512:(dc + 1) * 512], po[:], xo[:])
                dma(x2buf[it * 128:(it + 1) * 128, :], x2[:])
                h2 = h2_p.tile([128, D], BF16)
                layernorm_tile(stat, scratch, x2, h2)
                for dt in range(DT):
                    tp = tps.tile([128, 128], BF16)
                    nc.tensor.transpose(
                        tp[:], h2[:, dt * 128:(dt + 1) * 128], ident[:])
                    nc.vector.tensor_copy(
                        h2_fm[:, dt * OWN + it * 128:
                              dt * OWN + (it + 1) * 128],
                        tp[:])

        es_c.close()

        # ---- phase F1: MLP up-proj + GELU -> gbuf ------------------------
        with ExitStack() as ph:
            w1_p = ph.enter_context(tc.tile_pool(name="w1", bufs=3))
            gst_p = ph.enter_context(tc.tile_pool(name="gst", bufs=3))
            ps_m = ph.enter_context(
                tc.tile_pool(name="psm", bufs=2, space="PSUM"))
            W1C = min(512, OWN)
            for ft in range(FT):
                w1t = w1_p.tile([128, DT * 128], BF16)
                dma_blk(w1t, w1_d[:, ft * 128:(ft + 1) * 128])
                for oc in range(OWN // W1C):
                    ps = ps_m.tile([128, W1C], F32)
                    for dt in range(DT):
                        nc.tensor.matmul(
                            ps[:],
                            w1t[:, dt * 128:(dt + 1) * 128],
                            h2_fm[:, dt * OWN + oc * W1C:
                                  dt * OWN + (oc + 1) * W1C],
                            start=(dt == 0), stop=(dt == DT - 1))
                    gt = gst_p.tile([128, W1C], BF16)
                    nc.scalar.activation(gt[:], ps[:], GELU,
                                         bias=b1c[:, ft:ft + 1], scale=1.0)
                    dma(
                        gbuf[ft * 128:(ft + 1) * 128,
                             oc * W1C:(oc + 1) * W1C], gt[:])

        es_b.close()

        # ---- phase F2: MLP down-proj + residual -> out -------------------
        with ExitStack() as ph:
            gs_p = ph.enter_context(tc.tile_pool(name="gs", bufs=2))
            b2_p = ph.enter_context(tc.tile_pool(name="b2", bufs=1))
            x2s_p = ph.enter_context(tc.tile_pool(name="x2s", bufs=3))
            o_p = ph.enter_context(tc.tile_pool(name="osb", bufs=3))
            ps_m = ph.enter_context(
                tc.tile_pool(name="psm2", bufs=2, space="PSUM"))
            b2row = b2_p.tile([2, D], BF16)
            dma(b2row[:], b2row_d[:])
            for dc in range(D // 512):
                w2t = pool_a.tile([128, FT * 512], BF16, tag="a")
                dma_blk(w2t, w2_d[:, dc * 512:(dc + 1) * 512])
                for it in range(OTT):
                    gs = gs_p.tile([128, FT * 128], BF16)
                    dma_blk(gs, gbuf[:, it * 128:(it + 1) * 128])
                    ps = ps_m.tile([128, 512], F32)
                    nc.tensor.matmul(
                        ps[:], ones2[:], b2row[:, dc * 512:(dc + 1) * 512],
                        start=True, stop=False)
                    for ft in range(FT):
                        nc.tensor.matmul(
                            ps[:],
                            gs[:, ft * 128:(ft + 1) * 128],
                            w2t[:, ft * 512:(ft + 1) * 512],
                            start=False, stop=(ft == FT - 1))
                    x2t = x2s_p.tile([128, 512], F32)
                    dma(
                        x2t[:],
                        x2buf[it * 128:(it + 1) * 128,
                              dc * 512:(dc + 1) * 512])
                    ot = o_p.tile([128, 512], F32)
                    nc.vector.tensor_add(ot[:], ps[:], x2t[:])
                    dma(
                        out_d[it * 128:(it + 1) * 128,
                              dc * 512:(dc + 1) * 512],
                        ot[:])

    _legalize_waits(nc)
    return nc


def _legalize_waits(nc):
    """walrus on this container encodes at most ONE sync wait per DMA/branch
    instruction. Tile emits several (reader-WAR + DMA-lane WAW). Waits are
    executed by the issuing engine's sequencer in program order, so hoisting
    all-but-one wait onto wait-only EventSemaphore instructions inserted
    immediately before the offender is semantics-preserving."""
    n_split = 0
    for fn in nc.m.functions:
        for bb in fn.blocks:
            out = []
            for inst in bb.instructions:
                si = inst.sync_info
                waits = list(si.on_wait) if si and si.on_wait else []
                if len(waits) > 1:
                    # merge same-sem waits to the max value
                    merged = {}
                    for w in waits:
                        k = (w.sync_type, w.id, w.wait_mode)
                        if k not in merged or merged[k].wait_value < w.wait_value:
                            merged[k] = w
                    waits = list(merged.values())
                    for w in waits[:-1]:
                        es = mybir.InstEventSemaphore(
                            name=f"{inst.name}-wsplit{n_split}",
                            engine=inst.engine,
                            ins=[], outs=[],
                            sync_info=mybir.SyncInfo(on_wait=[w], on_update=[]),
                        )
                        out.append(es)
                        n_split += 1
                    inst.sync_info = mybir.SyncInfo(
                        on_wait=[waits[-1]],
                        on_update=list(si.on_update) if si.on_update else [])
                out.append(inst)
            bb.instructions[:] = out


# ------------------------------------------------------------- the entry ---

_BUILT = {}


def _get_nc(cfg_key=None):
    if "nc" not in _BUILT:
        _BUILT["nc"] = build_kernel(REAL_CFG)
    return _BUILT["nc"]


def kernel(**inputs):
    cfg = REAL_CFG
    c = _cfg_derived(cfg)
    nc = _get_nc()
    in_maps = [make_core_inputs(cfg, inputs, core) for core in range(8)]
    from concourse.bass_utils import run_bass_kernel_spmd
    res = run_bass_kernel_spmd(nc, in_maps, list(range(8)))
    B = np.asarray(inputs["x"]).shape[0]
    S, D, OWN = cfg["S"], cfg["D"], c["OWN"]
    out = np.empty((B, S, D), np.float32)
    for core in range(8):
        b, r = core // 2, core % 2
        out[b, r * OWN:(r + 1) * OWN, :] = res.results[core]["out"]
    return out



# revision 35
# speedup vs baseline: 34.7782x; 34.7782x over previous
"""Trainium2 Bass kernel for a pre-LN transformer block (dense_transformer).

Reference computation (fp32, per batch element):
    x = x + Attn(LN1(x));  x = x + MLP(LN2(x))
with 16-head causal ALiBi attention (S=2048, D=2048) and a 4*D GELU MLP.

Distribution: 4 batches x 2-way head/tensor parallel = 8 cores.
Core c handles batch c//2 with pair-rank r=c%2:
  - attention: 8 local heads (r*8..r*8+7), all 2048 query positions. Scores
    are computed transposed [j(key) x i(query)] so the ALiBi +s_h*j term rides
    the ACT-exp per-partition bias and the -s_h*i term is a K=3 seed matmul
    (bf16 hi/lo/lolo rows reconstruct the fp32 column bias in PSUM). The
    combined exp argument qk/sqrt(hd) + s_h*(j-i) is <= O(1), so no running
    max is needed; per-query softmax scale factors cancel in the normalize.
  - softmax denominators via ones-lhsT matmuls accumulated alongside AV;
    normalization is fused into the AV PSUM->SBUF copy using a K=1 broadcast
    matmul of the reciprocal row.
  - a pair AllGather swaps attention halves so each core owns 1024 tokens
    for the output projection, residual, LN2 and MLP (full 4*D hidden); a
    data-driven select (sel input) combines the two token-halves from the
    gather output so the instruction stream stays rank-agnostic.
All per-core variation (weight slices, ALiBi slopes, token offsets) is input
DATA; the instruction stream is identical on all 8 cores (SPMD).

DMA queues: weight streams ride the SP HWDGE queue, activation loads the
Activation HWDGE queue, and stores/exchange the gpsimd SWDGE queue, so the
three traffic classes move in parallel instead of serializing on one ring.

The walrus in this container encodes at most ONE sync wait per instruction,
so _legalize_waits() splits every multi-wait instruction into wait-only
EventSemaphore instructions inserted immediately before it on the same
engine stream - order-preserving, so semantics are unchanged.
"""

import os
import sys

for _p in ("/opt/trn_rl_repo", "/opt/trn_rl_repo/concourse"):
    if os.path.isdir(_p) and _p not in sys.path:
        sys.path.append(_p)

import numpy as np
import ml_dtypes

import concourse.bass as bass
import concourse.mybir as mybir
import concourse.tile as tile
from contextlib import ExitStack

BF16 = mybir.dt.bfloat16
F32 = mybir.dt.float32
AF = mybir.ActivationFunctionType
ALU = mybir.AluOpType

REAL_CFG = dict(S=2048, D=2048, F=8192, H=16, GELU="Gelu")
LN_EPS = 1e-5
NEG = -1.0e6  # causal mask additive value (pre-exp)


def _cfg_derived(cfg):
    S, D, F, H = cfg["S"], cfg["D"], cfg["F"], cfg["H"]
    d = dict(cfg)
    d["HL"] = H // 2              # local heads per core
    d["HLW"] = d["HL"] * 128      # local head width (vd)
    d["DT"] = D // 128
    d["ST"] = S // 128
    d["QW"] = 512                 # q-chunk width (asserted below)
    d["CQ"] = S // 512
    d["OWN"] = S // 2
    d["OTT"] = d["OWN"] // 128
    d["FT"] = F // 128
    d["VDT"] = H
    assert S % 512 == 0 and D % 512 == 0 and F % 512 == 0
    return d


# ------------------------------------------------------------ host prep ---


def _bf(x):
    return np.asarray(x, np.float32).astype(ml_dtypes.bfloat16)


def _split3(v):
    """Split fp32 array (last axis vectors) into 3 bf16 rows summing to it."""
    v = np.asarray(v, np.float32)
    r0 = v.astype(ml_dtypes.bfloat16)
    rem = v - r0.astype(np.float32)
    r1 = rem.astype(ml_dtypes.bfloat16)
    r2 = (rem - r1.astype(np.float32)).astype(ml_dtypes.bfloat16)
    return np.stack([r0, r1, r2])


def make_core_inputs(cfg, inputs, core):
    c = _cfg_derived(cfg)
    S, D, F, H, HL, ST = c["S"], c["D"], c["F"], c["H"], c["HL"], c["ST"]
    HLW, OWN, FT = c["HLW"], c["OWN"], c["FT"]
    b, r = core // 2, core % 2
    hd = 128
    f32 = np.float32

    x = np.asarray(inputs["x"][b], f32)
    g1 = np.asarray(inputs["ln1_w"], f32)
    c1 = np.asarray(inputs["ln1_b"], f32)
    g2 = np.asarray(inputs["ln2_w"], f32)
    c2 = np.asarray(inputs["ln2_b"], f32)
    Wqkv = np.asarray(inputs["Wqkv"], f32)
    bqkv = np.asarray(inputs["bqkv"], f32)
    Wo = np.asarray(inputs["Wo"], f32)
    bo = np.asarray(inputs["bo"], f32)
    W1 = np.asarray(inputs["W1"], f32)
    b1 = np.asarray(inputs["b1"], f32)
    W2 = np.asarray(inputs["W2"], f32)
    b2 = np.asarray(inputs["b2"], f32)
    slopes = np.asarray(inputs["slopes"], f32)

    Wq, Wk, Wv = Wqkv[:D], Wqkv[D:2 * D], Wqkv[2 * D:]
    bq, bk, bv = bqkv[:D], bqkv[D:2 * D], bqkv[2 * D:]

    lo, hi = r * HLW, (r + 1) * HLW
    sc = 1.0 / np.sqrt(hd)

    Wq_l = Wq[lo:hi] * g1[None, :]
    Wk_l = Wk[lo:hi] * g1[None, :]
    Wv_l = Wv[lo:hi] * g1[None, :]
    qb = (Wq[lo:hi] @ c1 + bq[lo:hi]) * sc
    kb = Wk[lo:hi] @ c1 + bk[lo:hi]
    cbo = Wo @ (Wv @ c1 + bv) + bo          # v-bias + bo folded constant [D]

    W1p = W1 * g2[None, :]
    b1p = W1 @ c2 + b1

    heads = np.arange(r * HL, r * HL + HL)
    sl = slopes[heads]
    jpos = np.arange(S, dtype=f32)

    expbias = np.zeros((128, HL * ST), f32)
    for h in range(HL):
        for jt in range(ST):
            expbias[:, h * ST + jt] = sl[h] * (jt * 128 + jpos[:128])

    seed = np.zeros((3, HL * S), f32)
    for h in range(HL):
        seed[:, h * S:(h + 1) * S] = _split3(-sl[h] * jpos)

    masks = np.zeros((128, 4 * 512), f32)
    jj = np.arange(128)[:, None]
    ii = np.arange(512)[None, :]
    for m in range(4):
        masks[:, m * 512:(m + 1) * 512] = np.where(m * 128 + jj <= ii, 0.0, NEG)

    return {
        "x_full": np.ascontiguousarray(x),
        "x_own": np.ascontiguousarray(x[r * OWN:(r + 1) * OWN]),
        "wq_t": np.ascontiguousarray(_bf(Wq_l.T * sc)),
        "wk_t": np.ascontiguousarray(_bf(Wk_l.T)),
        "wv_t": np.ascontiguousarray(_bf(Wv_l.T)),
        "wo_t": np.ascontiguousarray(_bf(Wo.T)),
        "w1_t": np.ascontiguousarray(_bf(W1p.T)),
        "w2_t": np.ascontiguousarray(_bf(W2.T)),
        "qb": np.ascontiguousarray(qb.reshape(HL, 128).T),
        "kb": np.ascontiguousarray(kb.reshape(HL, 128).T),
        "b1c": np.ascontiguousarray(b1p.reshape(FT, 128).T),
        "obias": np.ascontiguousarray(_split3(cbo)[:2]),
        "b2row": np.ascontiguousarray(_split3(b2)[:2]),
        "expbias": expbias,
        "seed": np.ascontiguousarray(seed.astype(ml_dtypes.bfloat16)),
        "masks": np.ascontiguousarray(masks.astype(ml_dtypes.bfloat16)),
        "ident": np.eye(128, dtype=f32).astype(ml_dtypes.bfloat16),
        "sel": np.ascontiguousarray(
            np.repeat((np.arange(2) == r).astype(f32)[None, :], 128, axis=0)),
    }


# ------------------------------------------------------------ the kernel ---


def build_kernel(cfg):
    c = _cfg_derived(cfg)
    S, D, F = c["S"], c["D"], c["F"]
    HL, HLW, DT, ST = c["HL"], c["HLW"], c["DT"], c["ST"]
    CQ, QW, OWN, OTT, FT, VDT = (c["CQ"], c["QW"], c["OWN"], c["OTT"],
                                 c["FT"], c["VDT"])
    GELU = getattr(AF, cfg["GELU"])

    # Single SWDGE sem lane: the walrus DMA ISA encodes at most 2 waits;
    # Tile's default 8-lane round-robin sem assignment makes slot-reuse DMAs
    # wait on several DMASW sems at once. _legalize_waits splits the rest.
    import concourse.tile_sem_assignment as tsa
    tsa.NUM_SWDGE_GLOBAL_SEMS = 1

    nc = bass.Bass()

    def param(name, shape, dt):
        return nc.declare_dram_parameter(name, shape, dt, isOutput=False)

    x_full_d = param("x_full", [S, D], F32)
    x_own_d = param("x_own", [OWN, D], F32)
    wq_d = param("wq_t", [D, HLW], BF16)
    wk_d = param("wk_t", [D, HLW], BF16)
    wv_d = param("wv_t", [D, HLW], BF16)
    wo_d = param("wo_t", [D, D], BF16)
    w1_d = param("w1_t", [D, F], BF16)
    w2_d = param("w2_t", [F, D], BF16)
    qb_d = param("qb", [128, HL], F32)
    kb_d = param("kb", [128, HL], F32)
    b1c_d = param("b1c", [128, FT], F32)
    obias_d = param("obias", [2, D], BF16)
    b2row_d = param("b2row", [2, D], BF16)
    expb_d = param("expbias", [128, HL * ST], F32)
    seed_d = param("seed", [3, HL * S], BF16)
    masks_d = param("masks", [128, 4 * 512], BF16)
    ident_d = param("ident", [128, 128], BF16)
    sel_d = param("sel", [128, 2], F32)
    out_d = nc.declare_dram_parameter("out", [OWN, D], F32, isOutput=True)

    groups = [[0, 1], [2, 3], [4, 5], [6, 7]]

    with tile.TileContext(nc) as tc, ExitStack() as top:
        def dma(out_ap, in_ap):
            nc.gpsimd.dma_start(out_ap, in_ap)

        def dma_w(out_ap, in_ap):
            nc.sync.dma_start(out_ap, in_ap)        # SP HWDGE: weights

        def dma_a(out_ap, in_ap):
            nc.scalar.dma_start(out_ap, in_ap)      # Act HWDGE: activations

        def _blk(fn, sbuf_tile, dram_ap):
            """DMA DRAM [T*128, N] into SBUF [128, T*N] (block t at t*N)."""
            rows = dram_ap.shape[0]
            t = rows // 128
            fn(sbuf_tile[:].rearrange("p (t f) -> p t f", t=t),
               dram_ap.rearrange("(t p) f -> p t f", p=128))

        def dma_blk_w(sbuf_tile, dram_ap):
            _blk(dma_w, sbuf_tile, dram_ap)

        def dma_blk_a(sbuf_tile, dram_ap):
            _blk(dma_a, sbuf_tile, dram_ap)

        dram = top.enter_context(tc.tile_pool(name="dram", bufs=1,
                                              space="DRAM"))
        exch_in = dram.tile([2, HLW, OWN], BF16)
        ago = dram.tile([2, 2, HLW, OWN], BF16)
        x2buf = dram.tile([OWN, D], F32)
        gbuf = dram.tile([F, OWN], BF16)
        const = top.enter_context(tc.tile_pool(name="const", bufs=1))
        ident = const.tile([128, 128], BF16)
        expb = const.tile([128, HL * ST], F32)
        qb = const.tile([128, HL], F32)
        kb = const.tile([128, HL], F32)
        b1c = const.tile([128, FT], F32)
        ones3 = const.tile([3, 128], BF16)
        ones2 = const.tile([2, 128], BF16)
        onesc = const.tile([128, 1], BF16)
        onesr = const.tile([1, 128], BF16)
        epsc = const.tile([128, 1], F32)
        sel = const.tile([128, 2], F32)

        dma_w(ident[:], ident_d[:])
        dma_w(qb[:], qb_d[:])
        dma_w(kb[:], kb_d[:])
        dma_w(expb[:], expb_d[:])
        dma_w(b1c[:], b1c_d[:])
        dma_w(sel[:], sel_d[:])
        nc.vector.memset(ones3[:], 1.0)
        nc.vector.memset(ones2[:], 1.0)
        nc.vector.memset(onesc[:], 1.0)
        nc.vector.memset(onesr[:], 1.0)
        nc.vector.memset(epsc[:], LN_EPS)

        # Slot pools enter in reverse order of death (Tile pools release
        # LIFO): B (k_sb -> h2_fm) dies last, after the MLP up-projection;
        # A (h_fm -> wo_sb) and C (v_sb -> attg) die after phase E; D
        # (streamed qkv weights) dies after attention.
        es_a, es_b, es_c, es_d = (ExitStack(), ExitStack(), ExitStack(),
                                  ExitStack())
        pool_b = es_b.enter_context(tc.tile_pool(name="slotB", bufs=1))
        pool_a = es_a.enter_context(tc.tile_pool(name="slotA", bufs=1))

        # ---- LayerNorm helper (normalized output only; w/b pre-folded) ---
        LNG = D // 512 if D >= 512 else 1

        def layernorm_tile(stat, xt, out_bf):
            st6 = stat.tile([128, 6 * LNG], F32)
            for g in range(LNG):
                nc.vector.bn_stats(st6[:, 6 * g:6 * (g + 1)],
                                   xt[:, 512 * g:512 * (g + 1)])
            ag = stat.tile([128, 2], F32)
            nc.vector.bn_aggr(ag[:], st6[:])
            sd = stat.tile([128, 1], F32)
            nc.scalar.activation(sd[:], ag[:, 1:2], AF.Sqrt,
                                 bias=epsc[:], scale=1.0)
            r = stat.tile([128, 1], F32)
            nc.vector.reciprocal(r[:], sd[:])
            nc.vector.tensor_scalar(
                out_bf[:], xt[:], scalar1=ag[:, 0:1], scalar2=r[:],
                op0=ALU.subtract, op1=ALU.mult)

        # ---- phase A: LN1 + transpose into h_fm ------------------------
        h_fm = pool_a.tile([128, DT * S], BF16, tag="a")
        with ExitStack() as ph:
            xpool = ph.enter_context(tc.tile_pool(name="ln_x", bufs=2))
            stat = ph.enter_context(tc.tile_pool(name="ln_stat", bufs=8))
            hbf = ph.enter_context(tc.tile_pool(name="ln_h", bufs=2))
            tps = ph.enter_context(
                tc.tile_pool(name="tps", bufs=6, space="PSUM"))
            for tt in range(ST):
                xt = xpool.tile([128, D], F32)
                dma_a(xt[:], x_full_d[tt * 128:(tt + 1) * 128, :])
                ht = hbf.tile([128, D], BF16)
                layernorm_tile(stat, xt, ht)
                for dt in range(DT):
                    tp = tps.tile([128, 128], BF16)
                    nc.tensor.transpose(
                        tp[:], ht[:, dt * 128:(dt + 1) * 128], ident[:])
                    nc.vector.tensor_copy(
                        h_fm[:, dt * S + tt * 128: dt * S + (tt + 1) * 128],
                        tp[:])

        # ---- phase C1: K and V projections -------------------------------
        pool_c = es_c.enter_context(tc.tile_pool(name="slotC", bufs=1))
        pool_d = es_d.enter_context(tc.tile_pool(name="slotD", bufs=1))
        k_sb = pool_b.tile([128, HL * S], BF16, tag="b")
        v_sb = pool_c.tile([128, ST * HLW], BF16, tag="c")
        with ExitStack() as ph:
            mps = ph.enter_context(
                tc.tile_pool(name="c1ps", bufs=3, space="PSUM"))
            wk_sb = pool_d.tile([128, DT * HLW], BF16, tag="d")
            dma_blk_w(wk_sb, wk_d.ap())
            for h in range(HL):
                for ch in range(CQ):
                    ps = mps.tile([128, QW], F32)
                    for dt in range(DT):
                        nc.tensor.matmul(
                            ps[:],
                            wk_sb[:, dt * HLW + h * 128:
                                  dt * HLW + (h + 1) * 128],
                            h_fm[:, dt * S + ch * QW: dt * S + (ch + 1) * QW],
                            start=(dt == 0), stop=(dt == DT - 1))
                    nc.vector.tensor_scalar_add(
                        k_sb[:, h * S + ch * QW: h * S + (ch + 1) * QW],
                        ps[:], kb[:, h:h + 1])
            wv_sb = pool_d.tile([128, DT * HLW], BF16, tag="d")
            dma_blk_w(wv_sb, wv_d.ap())
            VCW = min(512, HLW)
            for jt in range(ST):
                for vc in range(HLW // VCW):
                    ps = mps.tile([128, VCW], F32)
                    for dt in range(DT):
                        nc.tensor.matmul(
                            ps[:],
                            h_fm[:, dt * S + jt * 128: dt * S + (jt + 1) * 128],
                            wv_sb[:, dt * HLW + vc * VCW:
                                  dt * HLW + (vc + 1) * VCW],
                            start=(dt == 0), stop=(dt == DT - 1))
                    nc.vector.tensor_copy(
                        v_sb[:, jt * HLW + vc * VCW:
                             jt * HLW + (vc + 1) * VCW],
                        ps[:])

        # ---- phase C2: attention -----------------------------------------
        wq_sb = pool_d.tile([128, DT * HLW], BF16, tag="d")
        dma_blk_w(wq_sb, wq_d.ap())
        with ExitStack() as ph:
            msk_p = ph.enter_context(tc.tile_pool(name="mskp", bufs=1))
            masks = msk_p.tile([128, 4 * 512], BF16)
            dma_w(masks[:], masks_d[:])
            qc_p = ph.enter_context(tc.tile_pool(name="qc", bufs=2))
            seed_p = ph.enter_context(tc.tile_pool(name="seedp", bufs=2))
            att_p = ph.enter_context(tc.tile_pool(name="att", bufs=3))
            bcn_p = ph.enter_context(tc.tile_pool(name="bcn", bufs=2))
            den_p = ph.enter_context(tc.tile_pool(name="den", bufs=2))
            oat_p = ph.enter_context(tc.tile_pool(name="oat", bufs=2))
            ps_q = ph.enter_context(
                tc.tile_pool(name="psq", bufs=1, space="PSUM"))
            ps_s = ph.enter_context(
                tc.tile_pool(name="pss", bufs=2, space="PSUM"))
            ps_a = ph.enter_context(
                tc.tile_pool(name="psa", bufs=2, space="PSUM"))
            ps_d = ph.enter_context(
                tc.tile_pool(name="psd", bufs=2, space="PSUM"))
            ps_b = ph.enter_context(
                tc.tile_pool(name="psb", bufs=1, space="PSUM"))
            for ct in range(CQ):
                njt = min(ST, (ct + 1) * (QW // 128))
                for h in range(HL):
                    pq = ps_q.tile([128, QW], F32)
                    for dt in range(DT):
                        nc.tensor.matmul(
                            pq[:],
                            wq_sb[:, dt * HLW + h * 128:
                                  dt * HLW + (h + 1) * 128],
                            h_fm[:, dt * S + ct * QW: dt * S + (ct + 1) * QW],
                            start=(dt == 0), stop=(dt == DT - 1))
                    qc = qc_p.tile([128, QW], BF16)
                    nc.vector.tensor_scalar_add(qc[:], pq[:], qb[:, h:h + 1])
                    seedt = seed_p.tile([3, QW], BF16)
                    dma_a(
                        seedt[:],
                        seed_d[:, h * S + ct * QW: h * S + (ct + 1) * QW])

                    pav = ps_a.tile([128, QW], F32)
                    pden = ps_d.tile([1, QW], F32)
                    for jt in range(njt):
                        pss = ps_s.tile([128, QW], F32)
                        nc.tensor.matmul(
                            pss[:], ones3[:], seedt[:],
                            start=True, stop=False)
                        nc.tensor.matmul(
                            pss[:],
                            k_sb[:, h * S + jt * 128: h * S + (jt + 1) * 128],
                            qc[:], start=False, stop=True)
                        m = jt - ct * (QW // 128)
                        if 0 <= m < 4:
                            nc.vector.tensor_add(
                                pss[:], pss[:],
                                masks[:, m * 512: m * 512 + QW])
                        at = att_p.tile([128, QW], BF16)
                        nc.scalar.activation(
                            at[:], pss[:], AF.Exp,
                            bias=expb[:, h * ST + jt: h * ST + jt + 1],
                            scale=1.0)
                        nc.tensor.matmul(
                            pav[:],
                            v_sb[:, jt * HLW + h * 128:
                                 jt * HLW + (h + 1) * 128],
                            at[:], start=(jt == 0), stop=(jt == njt - 1))
                        nc.tensor.matmul(
                            pden[:], onesc[:], at[:],
                            start=(jt == 0), stop=(jt == njt - 1))
                    dsb = den_p.tile([1, QW], F32)
                    nc.vector.tensor_copy(dsb[:], pden[:])
                    rec = den_p.tile([1, QW], F32)
                    nc.vector.reciprocal(rec[:], dsb[:])
                    recb = den_p.tile([1, QW], BF16)
                    nc.vector.tensor_copy(recb[:], rec[:])
                    pbc = ps_b.tile([128, QW], F32)
                    nc.tensor.matmul(pbc[:], onesr[:], recb[:],
                                     start=True, stop=True)
                    bcn = bcn_p.tile([128, QW], F32)
                    nc.vector.tensor_copy(bcn[:], pbc[:])
                    oat = oat_p.tile([128, QW], BF16)
                    nc.vector.scalar_tensor_tensor(
                        oat[:], pav[:], 1.0, bcn[:],
                        op0=ALU.mult, op1=ALU.mult)
                    for half in range(2):
                        a = max(ct * QW, half * OWN)
                        bnd = min((ct + 1) * QW, (half + 1) * OWN)
                        if a < bnd:
                            dma(
                                exch_in[half, h * 128:(h + 1) * 128,
                                        a - half * OWN: bnd - half * OWN],
                                oat[:, a - ct * QW: bnd - ct * QW])

        es_d.close()

        # h_fm is dead once the last q-projection has read it: the out-proj
        # weight load (weight queue) overlaps the collective + gather DMAs.
        wo_sb = pool_a.tile([128, VDT * D], BF16, tag="a")
        dma_blk_w(wo_sb, wo_d.ap())

        # ---- phase D: pair exchange --------------------------------------
        nc.gpsimd.collective_compute(
            "AllGather", ALU.bypass, replica_groups=groups,
            ins=[exch_in.opt()], outs=[ago.opt()])

        # ---- phase E: out-proj + residual + LN2 + transpose --------------
        h2_fm = pool_b.tile([128, DT * OWN], BF16, tag="b")
        with ExitStack() as ph:
            agp = ph.enter_context(tc.tile_pool(name="agp", bufs=3))
            ob_p = ph.enter_context(tc.tile_pool(name="ob", bufs=1))
            xo_p = ph.enter_context(tc.tile_pool(name="xo", bufs=3))
            x2_p = ph.enter_context(tc.tile_pool(name="x2", bufs=2))
            h2_p = ph.enter_context(tc.tile_pool(name="h2", bufs=2))
            stat = ph.enter_context(tc.tile_pool(name="e_stat", bufs=8))
            ps_o = ph.enter_context(
                tc.tile_pool(name="pso", bufs=2, space="PSUM"))
            tps = ph.enter_context(
                tc.tile_pool(name="etps", bufs=6, space="PSUM"))

            attg = pool_c.tile([128, VDT * OWN], BF16, tag="c")
            for s in range(2):
                for h in range(HL):
                    g0 = agp.tile([128, OWN], BF16)
                    dma_a(g0[:], ago[s, 0, h * 128:(h + 1) * 128, :])
                    g1 = agp.tile([128, OWN], BF16)
                    dma_a(g1[:], ago[s, 1, h * 128:(h + 1) * 128, :])
                    t0 = agp.tile([128, OWN], BF16)
                    nc.vector.tensor_scalar_mul(t0[:], g0[:], sel[:, 0:1])
                    nc.vector.scalar_tensor_tensor(
                        attg[:, (s * HL + h) * OWN:(s * HL + h + 1) * OWN],
                        g1[:], sel[:, 1:2], t0[:],
                        op0=ALU.mult, op1=ALU.add)
            obias = ob_p.tile([2, D], BF16)
            dma_w(obias[:], obias_d[:])
            for it in range(OTT):
                x2 = x2_p.tile([128, D], F32)
                for dc in range(D // 512):
                    po = ps_o.tile([128, 512], F32)
                    nc.tensor.matmul(
                        po[:], ones2[:], obias[:, dc * 512:(dc + 1) * 512],
                        start=True, stop=False)
                    for v in range(VDT):
                        nc.tensor.matmul(
                            po[:],
                            attg[:, v * OWN + it * 128:
                                 v * OWN + (it + 1) * 128],
                            wo_sb[:, v * D + dc * 512: v * D + (dc + 1) * 512],
                            start=False, stop=(v == VDT - 1))
                    xo = xo_p.tile([128, 512], F32)
                    dma_a(
                        xo[:],
                        x_own_d[it * 128:(it + 1) * 128,
                                dc * 512:(dc + 1) * 512])
                    nc.vector.tensor_add(
                        x2[:, dc * 512:(dc + 1) * 512], po[:], xo[:])
                dma(x2buf[it * 128:(it + 1) * 128, :], x2[:])
                h2 = h2_p.tile([128, D], BF16)
                layernorm_tile(stat, x2, h2)
                for dt in range(DT):
                    tp = tps.tile([128, 128], BF16)
                    nc.tensor.transpose(
                        tp[:], h2[:, dt * 128:(dt + 1) * 128], ident[:])
                    nc.vector.tensor_copy(
                        h2_fm[:, dt * OWN + it * 128:
                              dt * OWN + (it + 1) * 128],
                        tp[:])

        es_c.close()
        es_a.close()

        # ---- phase F: MLP (up-proj + GELU -> gbuf, then down-proj) -------
        # F2's SBUF pools open before F1 emits so the first W2 piece and
        # b2row prefetch during F1 instead of stalling at the F1/F2 seam.
        FH = FT // 2
        W1C = min(512, OWN)
        with ExitStack() as ph:
            w1_p = ph.enter_context(tc.tile_pool(name="w1", bufs=3))
            gst_p = ph.enter_context(tc.tile_pool(name="gst", bufs=3))
            w2_p = ph.enter_context(tc.tile_pool(name="w2", bufs=2))
            gs_p = ph.enter_context(tc.tile_pool(name="gs", bufs=3))
            b2_p = ph.enter_context(tc.tile_pool(name="b2", bufs=1))
            x2s_p = ph.enter_context(tc.tile_pool(name="x2s", bufs=3))
            o_p = ph.enter_context(tc.tile_pool(name="osb", bufs=3))
            b2row = b2_p.tile([2, D], BF16)
            dma_w(b2row[:], b2row_d[:])

            # F1: up-projection + GELU
            with tc.tile_pool(name="psm", bufs=2, space="PSUM") as ps_m:
                for ft in range(FT):
                    w1t = w1_p.tile([128, DT * 128], BF16)
                    dma_blk_w(w1t, w1_d[:, ft * 128:(ft + 1) * 128])
                    for oc in range(OWN // W1C):
                        ps = ps_m.tile([128, W1C], F32)
                        for dt in range(DT):
                            nc.tensor.matmul(
                                ps[:],
                                w1t[:, dt * 128:(dt + 1) * 128],
                                h2_fm[:, dt * OWN + oc * W1C:
                                      dt * OWN + (oc + 1) * W1C],
                                start=(dt == 0), stop=(dt == DT - 1))
                        gt = gst_p.tile([128, W1C], BF16)
                        nc.scalar.activation(gt[:], ps[:], GELU,
                                             bias=b1c[:, ft:ft + 1],
                                             scale=1.0)
                        dma(
                            gbuf[ft * 128:(ft + 1) * 128,
                                 oc * W1C:(oc + 1) * W1C], gt[:])

            # F2: down-projection + residual. PSUM holds one [128,512]
            # accumulator per own-token tile (8 banks); W2 streams once in
            # [F/2, 512] pieces double-buffered on the weight queue, g^T
            # restreams from gbuf per (dc, fh, tt) on the act queue.
            ps_m2 = ph.enter_context(
                tc.tile_pool(name="psm2", bufs=OTT, space="PSUM"))
            for dc in range(D // 512):
                pss = []
                for it in range(OTT):
                    ps = ps_m2.tile([128, 512], F32)
                    nc.tensor.matmul(
                        ps[:], ones2[:], b2row[:, dc * 512:(dc + 1) * 512],
                        start=True, stop=False)
                    pss.append(ps)
                for fh in range(2):
                    w2t = w2_p.tile([128, FH * 512], BF16)
                    dma_blk_w(
                        w2t,
                        w2_d[fh * FH * 128:(fh + 1) * FH * 128,
                             dc * 512:(dc + 1) * 512])
                    for it in range(OTT):
                        gs = gs_p.tile([128, FH * 128], BF16)
                        dma_blk_a(
                            gs,
                            gbuf[fh * FH * 128:(fh + 1) * FH * 128,
                                 it * 128:(it + 1) * 128])
                        for ft in range(FH):
                            nc.tensor.matmul(
                                pss[it][:],
                                gs[:, ft * 128:(ft + 1) * 128],
                                w2t[:, ft * 512:(ft + 1) * 512],
                                start=False,
                                stop=(fh == 1 and ft == FH - 1))
                for it in range(OTT):
                    x2t = x2s_p.tile([128, 512], F32)
                    dma_a(
                        x2t[:],
                        x2buf[it * 128:(it + 1) * 128,
                              dc * 512:(dc + 1) * 512])
                    ot = o_p.tile([128, 512], F32)
                    nc.vector.tensor_add(ot[:], pss[it][:], x2t[:])
                    dma(
                        out_d[it * 128:(it + 1) * 128,
                              dc * 512:(dc + 1) * 512],
                        ot[:])

        es_b.close()

    _legalize_waits(nc)
    return nc


def _legalize_waits(nc):
    """walrus on this container encodes at most ONE sync wait per DMA/branch
    instruction. Tile emits several (reader-WAR + DMA-lane WAW). Waits are
    executed by the issuing engine's sequencer in program order, so hoisting
    all-but-one wait onto wait-only EventSemaphore instructions inserted
    immediately before the offender is semantics-preserving."""
    n_split = 0
    for fn in nc.m.functions:
        for bb in fn.blocks:
            out = []
            for inst in bb.instructions:
                si = inst.sync_info
                waits = list(si.on_wait) if si and si.on_wait else []
                if len(waits) > 1:
                    # merge same-sem waits to the max value
                    merged = {}
                    for w in waits:
                        k = (w.sync_type, w.id, w.wait_mode)
                        if k not in merged or merged[k].wait_value < w.wait_value:
                            merged[k] = w
                    waits = list(merged.values())
                    for w in waits[:-1]:
                        es = mybir.InstEventSemaphore(
                            name=f"{inst.name}-wsplit{n_split}",
                            engine=inst.engine,
                            ins=[], outs=[],
                            sync_info=mybir.SyncInfo(on_wait=[w], on_update=[]),
                        )
                        out.append(es)
                        n_split += 1
                    inst.sync_info = mybir.SyncInfo(
                        on_wait=[waits[-1]],
                        on_update=list(si.on_update) if si.on_update else [])
                out.append(inst)
            bb.instructions[:] = out
    return nc


# ------------------------------------------------------------- the entry ---

_BUILT = {}


def _get_nc(cfg_key=None):
    if "nc" not in _BUILT:
        _BUILT["nc"] = build_kernel(REAL_CFG)
    return _BUILT["nc"]


def kernel(**inputs):
    cfg = REAL_CFG
    c = _cfg_derived(cfg)
    nc = _get_nc()
    in_maps = [make_core_inputs(cfg, inputs, core) for core in range(8)]
    from concourse.bass_utils import run_bass_kernel_spmd
    res = run_bass_kernel_spmd(nc, in_maps, list(range(8)))
    B = np.asarray(inputs["x"]).shape[0]
    S, D, OWN = cfg["S"], cfg["D"], c["OWN"]
    out = np.empty((B, S, D), np.float32)
    for core in range(8):
        b, r = core // 2, core % 2
        out[b, r * OWN:(r + 1) * OWN, :] = res.results[core]["out"]
    return out
